# revision 41
# baseline (speedup 1.0000x reference)
"""Sparse-attention kernel for 8 trn2 NeuronCores (Bass/Tile).

Math (reference):
    Q = x1 @ Wq.T + bq                       [N1, DIM]
    K = x2 @ Wk.T + bk                       [N2, DIM]
    scores = (Q @ K.T) / sqrt(ITEM)          [N1, N2]
    e = exp(scores) * label_map
    att = e / (sum_j e + 1e-8) * (sum_j label_map / topk + 1e-8)
    out = att @ x2                           [N1, ITEM]

Key transformations used here:
  * Rows of x1/label_map are sharded across 8 cores (512 rows each).
  * The bk bias adds Q_i . bk to every score in row i; it scales both the
    numerator e and the denominator sum(e) by exp(c_i), which cancels in
    the normalization (the +1e-8 epsilon makes this inexact only at the
    ~1e-11 relative level since sum(e) is O(1e3)).  So bk drops out.
  * Each core projects only its own 512-column shard of K.T (1/8th of the
    33.5 GFLOP K projection); two chunked AllGathers (d-halves) assemble
    the full K.T while the rest of the K projection and the Q projection
    keep the tensor engine busy.
  * The 1/sqrt(ITEM) scale and bq bias are folded into the Q epilogue.
  * The per-row normalization a_i is applied to the final out rows, so the
    unnormalized e.T tiles (built via PE transposes) feed the spmm directly.
  * Matmul operands are bf16 (fp32 PSUM accumulation).
  * Every DRAM stream is host-rearranged partition-major so each SBUF slab
    loads with ONE fully contiguous DMA, and matmul operand tiles are
    free-dim slices of resident slabs.
  * DMA queue plan: the critical first-matmul inputs (wkt[0] on gpsimd,
    x2m[0] on scalar) are issued ahead of everything else; the bulk input
    stream (x1t, wqt, x2n, lm) rides the sync queue in consumption order;
    collective staging + K.T reload + y writeback ride scalar/vector.
  * ~10 warmup matmuls on the identity tile run during the DMA preamble so
    the PE HAM clock-gate is already released when real work arrives.
"""

import math

import numpy as np

try:
    import concourse.bass as bass
except ImportError:  # fresh interpreter without the boot path
    import sys

    sys.path.insert(0, "/opt/trn_rl_repo")
    import concourse.bass as bass

import ml_dtypes
import concourse.mybir as mybir
import concourse.tile as tile
from concourse import bacc
from concourse.bass_utils import run_bass_kernel_spmd
from concourse.masks import make_identity

NCORES = 8
F32 = mybir.dt.float32
BF16 = mybir.dt.bfloat16
NPBF16 = ml_dtypes.bfloat16


def _build(S, N2, ITEM, DIMP, denom, topk_f):
    """Build the per-core Bass program.

    S     - x1 rows per core (multiple of 128)
    N2    - x2 rows (multiple of 512)
    ITEM  - feature dim (multiple of 512)
    DIMP  - projection dim padded to a multiple of 128
    denom - sqrt(original ITEM)
    """
    IC = S // 128  # output-row chunks
    JC = N2 // 128  # x2-row chunks (spmm contraction)
    JN = N2 // 512  # 512-wide tiles of the scores free dim
    TC = ITEM // 128  # feature chunks (scores contraction)
    TN = ITEM // 512  # 512-wide tiles of the output free dim
    DC = DIMP // 128  # projection-dim chunks
    DH = DC // 2  # d-chunks in the first AllGather (symmetric halves)
    assert JN == NCORES, "requires N2 == 512 * NCORES"
    Exp = mybir.ActivationFunctionType.Exp
    Mult = mybir.AluOpType.mult
    Add = mybir.AluOpType.add
    X = mybir.AxisListType.X

    nc = bacc.Bacc("TRN2", target_bir_lowering=False, debug=False, num_devices=NCORES)
    # All streams are partition-major slabs: [slab_idx, 128, inner...] where
    # the per-partition inner block is contiguous in DRAM.
    x1t = nc.dram_tensor("x1t", [128, TC, S], BF16, kind="ExternalInput")
    wqt = nc.dram_tensor("wqt", [DC, 128, TC, 128], BF16, kind="ExternalInput")
    wkt = nc.dram_tensor("wkt", [DC, 128, TC, 128], BF16, kind="ExternalInput")
    x2m = nc.dram_tensor("x2m", [128, TC, 512], BF16, kind="ExternalInput")
    x2n = nc.dram_tensor("x2n", [TN, 128, JC, 512], BF16, kind="ExternalInput")
    lm = nc.dram_tensor("lm", [JN, 128, IC, 512], BF16, kind="ExternalInput")
    ints = nc.dram_tensor("ints", [128, IC], F32, kind="ExternalInput")
    bq2 = nc.dram_tensor("bq2", [128, DC], F32, kind="ExternalInput")
    y = nc.dram_tensor("y", [S, ITEM], F32, kind="ExternalOutput")

    with tile.TileContext(nc) as tc:
        with (
            tc.tile_pool(name="big", bufs=1) as big,
            tc.tile_pool(name="persist", bufs=1) as persist,
            tc.tile_pool(name="stream", bufs=4) as stream,
            tc.tile_pool(name="slab", bufs=12) as slabpool,
            tc.tile_pool(name="wq", bufs=4) as wqpool,
            tc.tile_pool(name="wq2", bufs=3) as wqpool2,
            tc.tile_pool(name="lmp", bufs=4) as lmpool,
            tc.tile_pool(name="ktp", bufs=3) as ktpool,
            tc.tile_pool(name="dram", bufs=1, space="DRAM") as drampool,
            tc.tile_pool(name="acc", bufs=6, space="PSUM") as accp,
            tc.tile_pool(name="trp", bufs=2, space="PSUM") as trp,
        ):
            # ---- critical-path DMAs first: wkt[0] (gpsimd) + x2m chunk 0
            # (scalar) gate the very first K-projection matmul.  wkt[1] rides
            # sync ahead of the bulk stream so K-proj d=1 never stalls.
            # x2m in eight 512KB chunks + first wkt slabs, spread across all
            # three DMA queues in need-time order so K-proj never starves.
            QWA = TC // 8
            wk0 = wqpool.tile([128, TC, 128], BF16, tag="wq", name="wkt_0")
            nc.sync.dma_start(wk0[:], wkt[0])
            wk1 = wqpool.tile([128, TC, 128], BF16, tag="wq", name="wkt_1")
            nc.gpsimd.dma_start(wk1[:], wkt[1])
            xmq = []
            xm_engines = [
                nc.scalar, nc.sync, nc.gpsimd, nc.scalar,
                nc.sync, nc.gpsimd, nc.scalar, nc.sync,
            ]
            for q in range(8):
                xs = slabpool.tile(
                    [128, QWA, 512], BF16, tag="slab", name=f"x2m_{q}"
                )
                xm_engines[q].dma_start(xs[:], x2m[:, q * QWA : (q + 1) * QWA, :])
                xmq.append(xs)
            wk2 = wqpool.tile([128, TC, 128], BF16, tag="wq", name="wkt_2")
            nc.sync.dma_start(wk2[:], wkt[2])
            # x1t for phase 1 streams on sync behind the critical chunks; the
            # scalar queue stays light so the collective staging writes
            # (ktin0/ktin1) fire with minimal latency.
            x1t_s = big.tile([128, TC, S], BF16, tag="bigA")
            nq = min(4, TC)
            qw = TC // nq
            for q in range(nq):
                nc.sync.dma_start(
                    x1t_s[:, q * qw : (q + 1) * qw, :], x1t[:, q * qw : (q + 1) * qw, :]
                )

            # ---- small constants (issued after the critical loads so they
            # don't head-of-line-block them).
            ident = persist.tile([128, 128], BF16, tag="ident")
            make_identity(nc, ident[:])
            zbias = persist.tile([128, 1], F32, tag="zbias")
            nc.gpsimd.memset(zbias[:], 0.0)
            bqt = persist.tile([128, DC], F32, tag="bqt")
            nc.gpsimd.dma_start(bqt[:], bq2[:])
            bqs = persist.tile([128, DC], F32, tag="bqs")
            nc.vector.tensor_scalar_mul(bqs[:], bqt[:], 1.0 / denom)

            # ---- PE warmup: release the HAM clock-gate while the first
            # input slabs stream in.  The results are never read.
            warm_ps = accp.tile([128, 512], F32, tag="acc", name="warm_ps")
            junk = persist.tile([128, 512], BF16, tag="junk")
            for w in range(14):
                nc.tensor.matmul(
                    warm_ps[:],
                    ident[:],
                    junk[:],
                    start=(w == 0),
                    stop=(w == 13),
                )
            nc.scalar.copy(junk[:], warm_ps[:])

            # phase A: K-shard projection KT_c[d, j_local], gathered in two
            # d-halves.  ("Shared" DRAM scratchpad is only pair-local on this
            # topology, so the collective engine must move the payload.)  Each
            # AllGather costs ~26µs entry handshake + rounds; both halves hide
            # under the rest of phase A and phase 1.
            ktsb = persist.tile([128, DC, 512], BF16, tag="ktsb")
            halves = [(0, DH), (DH, DC)]  # (2+6) asymmetric split
            ktin = [
                drampool.tile(
                    [128, h1 - h0, 512], BF16, tag=f"ktin{h}", name=f"ktin{h}"
                )
                for h, (h0, h1) in enumerate(halves)
            ]
            ktall = [
                drampool.tile(
                    [NCORES, 128, h1 - h0, 512], BF16,
                    tag=f"ktall{h}", name=f"ktall{h}", addr_space="Shared",
                )
                for h, (h0, h1) in enumerate(halves)
            ]
            wk_engines = {3: nc.scalar, 4: nc.gpsimd, 5: nc.gpsimd,
                          6: nc.gpsimd, 7: nc.gpsimd}
            for d in range(DC):
                if d == 0:
                    wsl = wk0
                elif d == 1:
                    wsl = wk1
                elif d == 2:
                    wsl = wk2
                else:
                    wsl = wqpool.tile([128, TC, 128], BF16, tag="wq", name=f"wkt_{d}")
                    wk_engines[d].dma_start(wsl[:], wkt[d])
                ps = accp.tile([128, 512], F32, tag="acc", name=f"psk_{d}")
                for t in range(TC):
                    nc.tensor.matmul(
                        ps[:],
                        wsl[:, t, :],
                        xmq[t // QWA][:, t % QWA, :],
                        start=(t == 0),
                        stop=(t == TC - 1),
                    )
                nc.vector.tensor_scalar_mul(ktsb[:, d, :], ps[:], 1.0)
                if d in (DH - 1, DC - 1):
                    h = 0 if d == DH - 1 else 1
                    h0, h1 = halves[h]
                    nc.scalar.dma_start(ktin[h][:], ktsb[:, h0:h1, :])
                    nc.gpsimd.collective_compute(
                        "AllGather",
                        mybir.AluOpType.bypass,
                        replica_groups=[list(range(NCORES))],
                        ins=[ktin[h][:].opt()],
                        outs=[ktall[h][:].opt()],
                    )

            # phase 1: QT[d, i] = (x1 @ Wq.T + bq) / denom, DIM-major
            qt_s = persist.tile([128, DC, S], BF16, tag="qt")
            for d in range(DC):
                wsl = wqpool2.tile([128, TC, 128], BF16, tag="wq2")
                nc.sync.dma_start(wsl[:], wqt[d])
                ps = accp.tile([128, 512], F32, tag="acc")
                for t in range(TC):
                    nc.tensor.matmul(
                        ps[:, :S],
                        wsl[:, t, :],
                        x1t_s[:, t, :],
                        start=(t == 0),
                        stop=(t == TC - 1),
                    )
                nc.vector.tensor_scalar(
                    qt_s[:, d, :], ps[:, :S], 1.0 / denom, bqs[:, d : d + 1],
                    op0=Mult, op1=Add,
                )

            # phase 3: scores -> exp -> *label -> row-sums -> transpose to eT
            # (interactions = row-sums of label_map come precomputed from the
            # host, saving one DVE reduction per tile.)
            et_s = big.tile([128, JC, IC * 128], BF16, tag="bigA")
            s_parts = persist.tile([128, IC, JN], F32, tag="sparts")
            cp_engines = [nc.scalar, nc.vector]
            for jn in range(JN):
                ktsl = ktpool.tile([128, DC, 512], BF16, tag="kts", name=f"kts_{jn}")
                nc.scalar.dma_start(ktsl[:, :DH, :], ktall[0][jn])
                nc.scalar.dma_start(ktsl[:, DH:, :], ktall[1][jn])
                lsl = lmpool.tile([128, IC, 512], BF16, tag="lmt")
                nc.gpsimd.dma_start(lsl[:], lm[jn])
                for i in range(IC):
                    ps = accp.tile([128, 512], F32, tag="acc", name=f"ps3_{jn}_{i}")
                    for d in range(DC):
                        nc.tensor.matmul(
                            ps[:],
                            qt_s[:, d, i * 128 : (i + 1) * 128],
                            ktsl[:, d, :],
                            start=(d == 0),
                            stop=(d == DC - 1),
                        )
                    e = stream.tile([128, 512], BF16, tag="e")
                    nc.scalar.activation(e[:], ps[:], Exp, bias=zbias[:])
                    nc.vector.tensor_mul(e[:], e[:], lsl[:, i, :])
                    nc.vector.reduce_sum(s_parts[:, i, jn : jn + 1], e[:], axis=X)
                    for jj in range(4):
                        pt = trp.tile([128, 128], BF16, tag="tr")
                        nc.tensor.transpose(
                            pt[:], e[:, jj * 128 : (jj + 1) * 128], ident[:]
                        )
                        eng = cp_engines[jj % 2]
                        if eng is nc.scalar:
                            eng.copy(
                                et_s[:, jn * 4 + jj, i * 128 : (i + 1) * 128], pt[:]
                            )
                        else:
                            eng.tensor_scalar_mul(
                                et_s[:, jn * 4 + jj, i * 128 : (i + 1) * 128],
                                pt[:], 1.0,
                            )

            # a_i = (interactions/topk + 1e-8) / (sum_e + 1e-8)
            s_all = persist.tile([128, IC, 1], F32, tag="sall")
            nc.vector.reduce_sum(s_all[:], s_parts[:], axis=X)
            nc.vector.tensor_scalar_add(s_all[:], s_all[:], 1e-8)
            rec = persist.tile([128, IC, 1], F32, tag="rec")
            nc.vector.reciprocal(rec[:], s_all[:])
            intt = persist.tile([128, IC], F32, tag="intt")
            nc.gpsimd.dma_start(intt[:], ints[:])
            i_all = persist.tile([128, IC, 1], F32, tag="iall")
            nc.vector.tensor_scalar(
                i_all[:, :, 0], intt[:], 1.0 / topk_f, 1e-8, op0=Mult, op1=Add
            )
            a_all = persist.tile([128, IC, 1], F32, tag="aall")
            nc.vector.tensor_mul(a_all[:], i_all[:], rec[:])

            # phase 4: out[i, :] = a_i * sum_j eT[j, i] * x2[j, :]
            QJ = JC // 8
            for n in range(TN):
                xq = []
                for q in range(8):
                    xs = slabpool.tile(
                        [128, QJ, 512], BF16, tag="slab", name=f"x2n_{n}_{q}"
                    )
                    eng = nc.sync if q % 2 == 0 else nc.gpsimd
                    eng.dma_start(xs[:], x2n[n, :, q * QJ : (q + 1) * QJ, :])
                    xq.append(xs)
                for i in range(IC):
                    ps = accp.tile([128, 512], F32, tag="acc", name=f"ps4_{n}_{i}")
                    for j in range(JC):
                        nc.tensor.matmul(
                            ps[:],
                            et_s[:, j, i * 128 : (i + 1) * 128],
                            xq[j // QJ][:, j % QJ, :],
                            start=(j == 0),
                            stop=(j == JC - 1),
                        )
                    o = stream.tile([128, 512], F32, tag="osb")
                    nc.vector.tensor_scalar_mul(o[:], ps[:], a_all[:, i, :])
                    nc.scalar.dma_start(
                        y[i * 128 : (i + 1) * 128, n * 512 : (n + 1) * 512], o[:]
                    )

    nc.compile()
    return nc


def _pmajor(a, p, inner):
    """[R, C] with R = nblk*p -> [p, nblk, inner...] partition-major, where
    each partition's inner block is contiguous."""
    R, C = a.shape
    nblk = R // p
    return np.ascontiguousarray(a.reshape(nblk, p, C).transpose(1, 0, 2))


def _in_maps(x1, x2, label_map, Wq, bq, Wk, DIMP, S):
    ITEM = x1.shape[1]
    N2 = x2.shape[0]
    DIM = Wq.shape[0]
    DC = DIMP // 128
    TC = ITEM // 128
    JN = N2 // 512
    TN = ITEM // 512
    JC = N2 // 128
    IC = S // 128

    wqp = np.zeros((DIMP, ITEM), NPBF16)
    wqp[:DIM] = Wq.astype(NPBF16)
    wkp = np.zeros((DIMP, ITEM), NPBF16)
    wkp[:DIM] = Wk.astype(NPBF16)
    bqp = np.zeros((DIMP,), np.float32)
    bqp[:DIM] = bq
    bq2 = np.ascontiguousarray(bqp.reshape(DC, 128).T)

    x1b = x1.astype(NPBF16)
    x2b = x2.astype(NPBF16)
    wqT = np.ascontiguousarray(wqp.T)  # [ITEM, DIMP]
    x2T = np.ascontiguousarray(x2b.T)  # [ITEM, N2]

    # wqt[d] = WqT[:, d-chunk] as [128, TC, 128] partition-major
    wqt = np.stack(
        [_pmajor(wqT[:, d * 128 : (d + 1) * 128], 128, None) for d in range(DC)]
    )
    # wkt[d] = WkT[:, d-chunk] as [128, TC, 128]
    wkT = np.ascontiguousarray(wkp.T)
    wktb = np.stack(
        [_pmajor(wkT[:, d * 128 : (d + 1) * 128], 128, None) for d in range(DC)]
    )
    # x2t[jn] = x2T[:, jn-chunk] as [128, TC, 512]
    x2tb = np.stack(
        [_pmajor(x2T[:, j * 512 : (j + 1) * 512], 128, None) for j in range(JN)]
    )
    # x2n[n] = x2[:, n-chunk] as [128, JC, 512]
    x2nb = np.stack(
        [_pmajor(x2b[:, n * 512 : (n + 1) * 512], 128, None) for n in range(TN)]
    )
    maps = []
    for c in range(NCORES):
        sl = slice(c * S, (c + 1) * S)
        shard = label_map[sl].astype(NPBF16)
        lmb = np.stack(
            [_pmajor(shard[:, j * 512 : (j + 1) * 512], 128, None) for j in range(JN)]
        )
        # interactions (label row sums); 0/1 entries so the f32 sum is exact
        intr = label_map[sl].astype(np.float32).sum(axis=1)  # [S]
        ints = np.ascontiguousarray(intr.reshape(IC, 128).T)  # [128, IC]
        maps.append(
            {
                "x1t": _pmajor(np.ascontiguousarray(x1b[sl].T), 128, None),
                "wqt": wqt,
                "wkt": wktb,
                "x2m": x2tb[c],
                "x2n": x2nb,
                "lm": lmb,
                "ints": ints,
                "bq2": bq2,
            }
        )
    return maps


def _run(x1, x2, label_map, Wq, bq, Wk, bk, topk, trace=False):
    x1 = np.asarray(x1, np.float32)
    x2 = np.asarray(x2, np.float32)
    label_map = np.asarray(label_map, np.float32)
    Wq = np.asarray(Wq, np.float32)
    bq = np.asarray(bq, np.float32)
    Wk = np.asarray(Wk, np.float32)
    N1, ITEM = x1.shape
    N2 = x2.shape[0]
    DIM = Wq.shape[0]
    S = N1 // NCORES
    DIMP = ((DIM + 127) // 128) * 128
    nc = _build(S, N2, ITEM, DIMP, math.sqrt(ITEM), float(topk))
    maps = _in_maps(x1, x2, label_map, Wq, bq, Wk, DIMP, S)
    res = run_bass_kernel_spmd(
        nc, maps, list(range(NCORES)), trace=trace, trace_cores=[0] if trace else None
    )
    out = np.concatenate([res.results[c]["y"] for c in range(NCORES)], axis=0)
    return out.astype(np.float32), res


def kernel(x1, x2, label_map, Wq, bq, Wk, bk, topk):
    out, _ = _run(x1, x2, label_map, Wq, bq, Wk, bk, topk)
    return out


# revision 42
# speedup vs baseline: 1.0251x; 1.0251x over previous
"""Sparse-attention kernel for 8 trn2 NeuronCores (Bass/Tile).

Math (reference):
    Q = x1 @ Wq.T + bq                       [N1, DIM]
    K = x2 @ Wk.T + bk                       [N2, DIM]
    scores = (Q @ K.T) / sqrt(ITEM)          [N1, N2]
    e = exp(scores) * label_map
    att = e / (sum_j e + 1e-8) * (sum_j label_map / topk + 1e-8)
    out = att @ x2                           [N1, ITEM]

Key transformations used here:
  * Rows of x1/label_map are sharded across 8 cores (512 rows each).
  * The bk bias adds Q_i . bk to every score in row i; it scales both the
    numerator e and the denominator sum(e) by exp(c_i), which cancels in
    the normalization (the +1e-8 epsilon makes this inexact only at the
    ~1e-11 relative level since sum(e) is O(1e3)).  So bk drops out.
  * Each core projects only its own 512-column shard of K.T (1/8th of the
    33.5 GFLOP K projection); two chunked AllGathers (d-halves) assemble
    the full K.T while the rest of the K projection and the Q projection
    keep the tensor engine busy.
  * The 1/sqrt(ITEM) scale and bq bias are folded into the Q epilogue.
  * The per-row normalization a_i is applied to the final out rows, so the
    unnormalized e.T tiles (built via PE transposes) feed the spmm directly.
  * Matmul operands are bf16 (fp32 PSUM accumulation).
  * Every DRAM stream is host-rearranged partition-major so each SBUF slab
    loads with ONE fully contiguous DMA, and matmul operand tiles are
    free-dim slices of resident slabs.
  * DMA queue plan: the critical first-matmul inputs (wkt[0] on gpsimd,
    x2m[0] on scalar) are issued ahead of everything else; the bulk input
    stream (x1t, wqt, x2n, lm) rides the sync queue in consumption order;
    collective staging + K.T reload + y writeback ride scalar/vector.
  * ~10 warmup matmuls on the identity tile run during the DMA preamble so
    the PE HAM clock-gate is already released when real work arrives.
"""

import math

import numpy as np

try:
    import concourse.bass as bass
except ImportError:  # fresh interpreter without the boot path
    import sys

    sys.path.insert(0, "/opt/trn_rl_repo")
    import concourse.bass as bass

import ml_dtypes
import concourse.mybir as mybir
import concourse.tile as tile
from concourse import bacc
from concourse.bass_utils import run_bass_kernel_spmd
from concourse.masks import make_identity

NCORES = 8
F32 = mybir.dt.float32
BF16 = mybir.dt.bfloat16
NPBF16 = ml_dtypes.bfloat16


def _build(S, N2, ITEM, DIMP, denom, topk_f):
    """Build the per-core Bass program.

    S     - x1 rows per core (multiple of 128)
    N2    - x2 rows (multiple of 512)
    ITEM  - feature dim (multiple of 512)
    DIMP  - projection dim padded to a multiple of 128
    denom - sqrt(original ITEM)
    """
    IC = S // 128  # output-row chunks
    JC = N2 // 128  # x2-row chunks (spmm contraction)
    JN = N2 // 512  # 512-wide tiles of the scores free dim
    TC = ITEM // 128  # feature chunks (scores contraction)
    TN = ITEM // 512  # 512-wide tiles of the output free dim
    DC = DIMP // 128  # projection-dim chunks
    DH = 3  # d-chunks in the first AllGather (3+5 split: earlier trigger)
    assert JN == NCORES, "requires N2 == 512 * NCORES"
    Exp = mybir.ActivationFunctionType.Exp
    Mult = mybir.AluOpType.mult
    Add = mybir.AluOpType.add
    X = mybir.AxisListType.X

    nc = bacc.Bacc("TRN2", target_bir_lowering=False, debug=False, num_devices=NCORES)
    # All streams are partition-major slabs: [slab_idx, 128, inner...] where
    # the per-partition inner block is contiguous in DRAM.
    x1t = nc.dram_tensor("x1t", [128, TC, S], BF16, kind="ExternalInput")
    wqt = nc.dram_tensor("wqt", [DC, 128, TC, 128], BF16, kind="ExternalInput")
    wkt = nc.dram_tensor("wkt", [DC, 128, TC, 128], BF16, kind="ExternalInput")
    x2m = nc.dram_tensor("x2m", [128, TC, 512], BF16, kind="ExternalInput")
    x2n = nc.dram_tensor("x2n", [TN, 128, JC, 512], BF16, kind="ExternalInput")
    lm = nc.dram_tensor("lm", [JN, 128, IC, 512], BF16, kind="ExternalInput")
    ints = nc.dram_tensor("ints", [128, IC], F32, kind="ExternalInput")
    bq2 = nc.dram_tensor("bq2", [128, DC], F32, kind="ExternalInput")
    y = nc.dram_tensor("y", [S, ITEM], F32, kind="ExternalOutput")

    with tile.TileContext(nc) as tc:
        with (
            tc.tile_pool(name="big", bufs=1) as big,
            tc.tile_pool(name="persist", bufs=1) as persist,
            tc.tile_pool(name="stream", bufs=4) as stream,
            tc.tile_pool(name="slab", bufs=12) as slabpool,
            tc.tile_pool(name="wq", bufs=4) as wqpool,
            tc.tile_pool(name="wq2", bufs=3) as wqpool2,
            tc.tile_pool(name="lmp", bufs=4) as lmpool,
            tc.tile_pool(name="ktp", bufs=3) as ktpool,
            tc.tile_pool(name="dram", bufs=1, space="DRAM") as drampool,
            tc.tile_pool(name="acc", bufs=6, space="PSUM") as accp,
            tc.tile_pool(name="trp", bufs=2, space="PSUM") as trp,
        ):
            # ---- critical-path DMAs first: wkt[0] (gpsimd) + x2m chunk 0
            # (scalar) gate the very first K-projection matmul.  wkt[1] rides
            # sync ahead of the bulk stream so K-proj d=1 never stalls.
            # x2m in eight 512KB chunks + first wkt slabs, spread across all
            # three DMA queues in need-time order so K-proj never starves.
            QWA = TC // 8
            wk0 = wqpool.tile([128, TC, 128], BF16, tag="wq", name="wkt_0")
            nc.sync.dma_start(wk0[:], wkt[0])
            wk1 = wqpool.tile([128, TC, 128], BF16, tag="wq", name="wkt_1")
            nc.gpsimd.dma_start(wk1[:], wkt[1])
            xmq = []
            xm_engines = [
                nc.scalar, nc.sync, nc.gpsimd, nc.scalar,
                nc.sync, nc.gpsimd, nc.scalar, nc.sync,
            ]
            for q in range(8):
                xs = slabpool.tile(
                    [128, QWA, 512], BF16, tag="slab", name=f"x2m_{q}"
                )
                xm_engines[q].dma_start(xs[:], x2m[:, q * QWA : (q + 1) * QWA, :])
                xmq.append(xs)
            wk2 = wqpool.tile([128, TC, 128], BF16, tag="wq", name="wkt_2")
            nc.sync.dma_start(wk2[:], wkt[2])
            # x1t for phase 1 streams on sync behind the critical chunks; the
            # scalar queue stays light so the collective staging writes
            # (ktin0/ktin1) fire with minimal latency.
            x1t_s = big.tile([128, TC, S], BF16, tag="bigA")
            nq = min(4, TC)
            qw = TC // nq
            for q in range(nq):
                nc.sync.dma_start(
                    x1t_s[:, q * qw : (q + 1) * qw, :], x1t[:, q * qw : (q + 1) * qw, :]
                )

            # ---- small constants (issued after the critical loads so they
            # don't head-of-line-block them).
            ident = persist.tile([128, 128], BF16, tag="ident")
            make_identity(nc, ident[:])
            zbias = persist.tile([128, 1], F32, tag="zbias")
            nc.gpsimd.memset(zbias[:], 0.0)
            bqt = persist.tile([128, DC], F32, tag="bqt")
            nc.gpsimd.dma_start(bqt[:], bq2[:])
            bqs = persist.tile([128, DC], F32, tag="bqs")
            nc.vector.tensor_scalar_mul(bqs[:], bqt[:], 1.0 / denom)

            # ---- PE warmup: release the HAM clock-gate while the first
            # input slabs stream in.  The results are never read.
            warm_ps = accp.tile([128, 512], F32, tag="acc", name="warm_ps")
            junk = persist.tile([128, 512], BF16, tag="junk")
            for w in range(14):
                nc.tensor.matmul(
                    warm_ps[:],
                    ident[:],
                    junk[:],
                    start=(w == 0),
                    stop=(w == 13),
                )
            nc.scalar.copy(junk[:], warm_ps[:])

            # phase A: K-shard projection KT_c[d, j_local], gathered in two
            # d-halves.  ("Shared" DRAM scratchpad is only pair-local on this
            # topology, so the collective engine must move the payload.)  Each
            # AllGather costs ~26µs entry handshake + rounds; both halves hide
            # under the rest of phase A and phase 1.
            ktsb = persist.tile([128, DC, 512], BF16, tag="ktsb")
            halves = [(0, DH), (DH, DC)]  # (2+6) asymmetric split
            ktin = [
                drampool.tile(
                    [128, h1 - h0, 512], BF16, tag=f"ktin{h}", name=f"ktin{h}"
                )
                for h, (h0, h1) in enumerate(halves)
            ]
            ktall = [
                drampool.tile(
                    [NCORES, 128, h1 - h0, 512], BF16,
                    tag=f"ktall{h}", name=f"ktall{h}", addr_space="Shared",
                )
                for h, (h0, h1) in enumerate(halves)
            ]
            wk_engines = {3: nc.scalar, 4: nc.gpsimd, 5: nc.gpsimd,
                          6: nc.gpsimd, 7: nc.gpsimd}
            for d in range(DC):
                if d == 0:
                    wsl = wk0
                elif d == 1:
                    wsl = wk1
                elif d == 2:
                    wsl = wk2
                else:
                    wsl = wqpool.tile([128, TC, 128], BF16, tag="wq", name=f"wkt_{d}")
                    wk_engines[d].dma_start(wsl[:], wkt[d])
                ps = accp.tile([128, 512], F32, tag="acc", name=f"psk_{d}")
                for t in range(TC):
                    nc.tensor.matmul(
                        ps[:],
                        wsl[:, t, :],
                        xmq[t // QWA][:, t % QWA, :],
                        start=(t == 0),
                        stop=(t == TC - 1),
                    )
                nc.vector.tensor_scalar_mul(ktsb[:, d, :], ps[:], 1.0)
                if d in (DH - 1, DC - 1):
                    h = 0 if d == DH - 1 else 1
                    h0, h1 = halves[h]
                    nc.scalar.dma_start(ktin[h][:], ktsb[:, h0:h1, :])
                    nc.gpsimd.collective_compute(
                        "AllGather",
                        mybir.AluOpType.bypass,
                        replica_groups=[list(range(NCORES))],
                        ins=[ktin[h][:].opt()],
                        outs=[ktall[h][:].opt()],
                    )

            # phase 1: QT[d, i] = (x1 @ Wq.T + bq) / denom, DIM-major
            qt_s = persist.tile([128, DC, S], BF16, tag="qt")
            for d in range(DC):
                wsl = wqpool2.tile([128, TC, 128], BF16, tag="wq2")
                nc.sync.dma_start(wsl[:], wqt[d])
                ps = accp.tile([128, 512], F32, tag="acc")
                for t in range(TC):
                    nc.tensor.matmul(
                        ps[:, :S],
                        wsl[:, t, :],
                        x1t_s[:, t, :],
                        start=(t == 0),
                        stop=(t == TC - 1),
                    )
                nc.vector.tensor_scalar(
                    qt_s[:, d, :], ps[:, :S], 1.0 / denom, bqs[:, d : d + 1],
                    op0=Mult, op1=Add,
                )

            # phase 3: scores -> exp -> *label -> row-sums -> transpose to eT
            # (interactions = row-sums of label_map come precomputed from the
            # host, saving one DVE reduction per tile.)
            et_s = big.tile([128, JC, IC * 128], BF16, tag="bigA")
            s_parts = persist.tile([128, IC, JN], F32, tag="sparts")
            cp_engines = [nc.scalar, nc.vector]
            for jn in range(JN):
                ktsl = ktpool.tile([128, DC, 512], BF16, tag="kts", name=f"kts_{jn}")
                nc.scalar.dma_start(ktsl[:, :DH, :], ktall[0][jn])
                nc.scalar.dma_start(ktsl[:, DH:, :], ktall[1][jn])
                lsl = lmpool.tile([128, IC, 512], BF16, tag="lmt")
                nc.gpsimd.dma_start(lsl[:], lm[jn])
                for i in range(IC):
                    ps = accp.tile([128, 512], F32, tag="acc", name=f"ps3_{jn}_{i}")
                    for d in range(DC):
                        nc.tensor.matmul(
                            ps[:],
                            qt_s[:, d, i * 128 : (i + 1) * 128],
                            ktsl[:, d, :],
                            start=(d == 0),
                            stop=(d == DC - 1),
                        )
                    e = stream.tile([128, 512], BF16, tag="e")
                    nc.scalar.activation(e[:], ps[:], Exp, bias=zbias[:])
                    nc.vector.tensor_mul(e[:], e[:], lsl[:, i, :])
                    nc.vector.reduce_sum(s_parts[:, i, jn : jn + 1], e[:], axis=X)
                    for jj in range(4):
                        pt = trp.tile([128, 128], BF16, tag="tr")
                        nc.tensor.transpose(
                            pt[:], e[:, jj * 128 : (jj + 1) * 128], ident[:]
                        )
                        eng = cp_engines[jj % 2]
                        if eng is nc.scalar:
                            eng.copy(
                                et_s[:, jn * 4 + jj, i * 128 : (i + 1) * 128], pt[:]
                            )
                        else:
                            eng.tensor_scalar_mul(
                                et_s[:, jn * 4 + jj, i * 128 : (i + 1) * 128],
                                pt[:], 1.0,
                            )

            # a_i = (interactions/topk + 1e-8) / (sum_e + 1e-8)
            s_all = persist.tile([128, IC, 1], F32, tag="sall")
            nc.vector.reduce_sum(s_all[:], s_parts[:], axis=X)
            nc.vector.tensor_scalar_add(s_all[:], s_all[:], 1e-8)
            rec = persist.tile([128, IC, 1], F32, tag="rec")
            nc.vector.reciprocal(rec[:], s_all[:])
            intt = persist.tile([128, IC], F32, tag="intt")
            nc.gpsimd.dma_start(intt[:], ints[:])
            i_all = persist.tile([128, IC, 1], F32, tag="iall")
            nc.vector.tensor_scalar(
                i_all[:, :, 0], intt[:], 1.0 / topk_f, 1e-8, op0=Mult, op1=Add
            )
            a_all = persist.tile([128, IC, 1], F32, tag="aall")
            nc.vector.tensor_mul(a_all[:], i_all[:], rec[:])

            # phase 4: out[i, :] = a_i * sum_j eT[j, i] * x2[j, :]
            QJ = JC // 8
            for n in range(TN):
                xq = []
                for q in range(8):
                    xs = slabpool.tile(
                        [128, QJ, 512], BF16, tag="slab", name=f"x2n_{n}_{q}"
                    )
                    eng = nc.sync if q % 2 == 0 else nc.gpsimd
                    eng.dma_start(xs[:], x2n[n, :, q * QJ : (q + 1) * QJ, :])
                    xq.append(xs)
                for i in range(IC):
                    ps = accp.tile([128, 512], F32, tag="acc", name=f"ps4_{n}_{i}")
                    for j in range(JC):
                        nc.tensor.matmul(
                            ps[:],
                            et_s[:, j, i * 128 : (i + 1) * 128],
                            xq[j // QJ][:, j % QJ, :],
                            start=(j == 0),
                            stop=(j == JC - 1),
                        )
                    o = stream.tile([128, 512], F32, tag="osb")
                    nc.vector.tensor_scalar_mul(o[:], ps[:], a_all[:, i, :])
                    nc.scalar.dma_start(
                        y[i * 128 : (i + 1) * 128, n * 512 : (n + 1) * 512], o[:]
                    )

    nc.compile()
    return nc


def _pmajor(a, p, inner):
    """[R, C] with R = nblk*p -> [p, nblk, inner...] partition-major, where
    each partition's inner block is contiguous."""
    R, C = a.shape
    nblk = R // p
    return np.ascontiguousarray(a.reshape(nblk, p, C).transpose(1, 0, 2))


def _in_maps(x1, x2, label_map, Wq, bq, Wk, DIMP, S):
    ITEM = x1.shape[1]
    N2 = x2.shape[0]
    DIM = Wq.shape[0]
    DC = DIMP // 128
    TC = ITEM // 128
    JN = N2 // 512
    TN = ITEM // 512
    JC = N2 // 128
    IC = S // 128

    wqp = np.zeros((DIMP, ITEM), NPBF16)
    wqp[:DIM] = Wq.astype(NPBF16)
    wkp = np.zeros((DIMP, ITEM), NPBF16)
    wkp[:DIM] = Wk.astype(NPBF16)
    bqp = np.zeros((DIMP,), np.float32)
    bqp[:DIM] = bq
    bq2 = np.ascontiguousarray(bqp.reshape(DC, 128).T)

    x1b = x1.astype(NPBF16)
    x2b = x2.astype(NPBF16)
    wqT = np.ascontiguousarray(wqp.T)  # [ITEM, DIMP]
    x2T = np.ascontiguousarray(x2b.T)  # [ITEM, N2]

    # wqt[d] = WqT[:, d-chunk] as [128, TC, 128] partition-major
    wqt = np.stack(
        [_pmajor(wqT[:, d * 128 : (d + 1) * 128], 128, None) for d in range(DC)]
    )
    # wkt[d] = WkT[:, d-chunk] as [128, TC, 128]
    wkT = np.ascontiguousarray(wkp.T)
    wktb = np.stack(
        [_pmajor(wkT[:, d * 128 : (d + 1) * 128], 128, None) for d in range(DC)]
    )
    # x2t[jn] = x2T[:, jn-chunk] as [128, TC, 512]
    x2tb = np.stack(
        [_pmajor(x2T[:, j * 512 : (j + 1) * 512], 128, None) for j in range(JN)]
    )
    # x2n[n] = x2[:, n-chunk] as [128, JC, 512]
    x2nb = np.stack(
        [_pmajor(x2b[:, n * 512 : (n + 1) * 512], 128, None) for n in range(TN)]
    )
    maps = []
    for c in range(NCORES):
        sl = slice(c * S, (c + 1) * S)
        shard = label_map[sl].astype(NPBF16)
        lmb = np.stack(
            [_pmajor(shard[:, j * 512 : (j + 1) * 512], 128, None) for j in range(JN)]
        )
        # interactions (label row sums); 0/1 entries so the f32 sum is exact
        intr = label_map[sl].astype(np.float32).sum(axis=1)  # [S]
        ints = np.ascontiguousarray(intr.reshape(IC, 128).T)  # [128, IC]
        maps.append(
            {
                "x1t": _pmajor(np.ascontiguousarray(x1b[sl].T), 128, None),
                "wqt": wqt,
                "wkt": wktb,
                "x2m": x2tb[c],
                "x2n": x2nb,
                "lm": lmb,
                "ints": ints,
                "bq2": bq2,
            }
        )
    return maps


def _run(x1, x2, label_map, Wq, bq, Wk, bk, topk, trace=False):
    x1 = np.asarray(x1, np.float32)
    x2 = np.asarray(x2, np.float32)
    label_map = np.asarray(label_map, np.float32)
    Wq = np.asarray(Wq, np.float32)
    bq = np.asarray(bq, np.float32)
    Wk = np.asarray(Wk, np.float32)
    N1, ITEM = x1.shape
    N2 = x2.shape[0]
    DIM = Wq.shape[0]
    S = N1 // NCORES
    DIMP = ((DIM + 127) // 128) * 128
    nc = _build(S, N2, ITEM, DIMP, math.sqrt(ITEM), float(topk))
    maps = _in_maps(x1, x2, label_map, Wq, bq, Wk, DIMP, S)
    res = run_bass_kernel_spmd(
        nc, maps, list(range(NCORES)), trace=trace, trace_cores=[0] if trace else None
    )
    out = np.concatenate([res.results[c]["y"] for c in range(NCORES)], axis=0)
    return out.astype(np.float32), res


def kernel(x1, x2, label_map, Wq, bq, Wk, bk, topk):
    out, _ = _run(x1, x2, label_map, Wq, bq, Wk, bk, topk)
    return out
